# revision 1
# baseline (speedup 1.0000x reference)
# Trainium2 Bass kernel for nn_CrossAttention (B=1, I=J=1024, C_S=1024,
# C_Z=128, H=16, D=64), sharded over the query dim i across 8 NeuronCores.
#
# Per-core program (i-slice of 128 query rows):
#   qT = (Wq s_c^T + bq)/sqrt(D)  kT = Wk k_in^T   v = k_in Wv^T  (bf16 matmuls)
#   z[j,i,h] = sum_c bias[i,j,c] Wz[c,h]   via PE-transpose of bias blocks
#              (bias cast to bf16 during DMA) followed by per-block matmuls
#   scoresT[j,i] = kT_h^T qT_h + z      (softmax over j = partition dim,
#              computed without max-subtraction; scores are O(1) here)
#   o[i,:] = sum_j exp(scoresT) * v_aug[j]  with v_aug carrying mask[j] in an
#              extra column so the denominator comes out of the same matmul
#   out = (sigmoid(s_c Wg^T) * o) @ Wo^T
#
# kernel(**inputs) takes FULL inputs, shards on host, runs SPMD on cores 0-7,
# gathers to the full [1, 1024, 1024] output.

import numpy as np

B, I, J, CS, CZ, H, D = 1, 1024, 1024, 1024, 128, 16, 64
NCORES = 8
NI = I // NCORES  # 128 query rows per core
P = 128

_last_results = None


def _build_program(skip=()):
    from contextlib import ExitStack

    import concourse.mybir as mybir
    import concourse.tile as tile
    from concourse import bacc
    from concourse.masks import make_identity

    f32 = mybir.dt.float32
    bf16 = mybir.dt.bfloat16
    AF = mybir.ActivationFunctionType
    ALU = mybir.AluOpType

    nc = bacc.Bacc("TRN2", target_bir_lowering=False, debug=False)

    # ---- dram io ----
    s_c = nc.dram_tensor("s_c", [NI, CS], f32, kind="ExternalInput").ap()
    bias_c = nc.dram_tensor("bias_c", [NI, J, CZ], f32, kind="ExternalInput").ap()
    k_in = nc.dram_tensor("k_in", [J, CS], f32, kind="ExternalInput").ap()
    mask = nc.dram_tensor("mask", [J], f32, kind="ExternalInput").ap()
    w_q = nc.dram_tensor("w_q", [CS, CS], f32, kind="ExternalInput").ap()
    w_k = nc.dram_tensor("w_k", [CS, CS], f32, kind="ExternalInput").ap()
    w_v = nc.dram_tensor("w_v", [CS, CS], f32, kind="ExternalInput").ap()
    w_g = nc.dram_tensor("w_g", [CS, CS], f32, kind="ExternalInput").ap()
    w_o = nc.dram_tensor("w_o", [CS, CS], f32, kind="ExternalInput").ap()
    b_q = nc.dram_tensor("b_q", [CS], f32, kind="ExternalInput").ap()
    w_z = nc.dram_tensor("w_z", [CZ, H], f32, kind="ExternalInput").ap()
    out_d = nc.dram_tensor("out", [NI, CS], f32, kind="ExternalOutput").ap()

    KC = CS // P  # 8 contraction chunks
    JC = J // P  # 8 key chunks

    with tile.TileContext(nc) as tc, ExitStack() as ctx:
        pool = lambda name, bufs: ctx.enter_context(tc.tile_pool(name=name, bufs=bufs))
        ppool = lambda name, bufs: ctx.enter_context(
            tc.tile_pool(name=name, bufs=bufs, space="PSUM")
        )

        const = pool("const", 1)
        wnat_p = pool("wnat", 2)
        wt_p = pool("wt", 2)
        wot_p = pool("wot", 1)
        kin_p = pool("kin", 1)
        small_p = pool("small", 1)
        big_p = pool("big", 1)
        bstage_p = pool("bstage", 4)
        bt_p = pool("bt", 3)
        st_p = pool("st", 2)
        et_p = pool("et", 2)
        r_p = pool("r", 2)
        outs_p = pool("outs", 1)

        tpsum = ppool("tpsum", 3)  # transpose targets (1 bank each)
        bigps = ppool("bigps", 2)  # [128,512] f32 matmul accumulators
        qkps = ppool("qkps", 2)
        ops = ppool("ops", 1)

        def copy_on(eng_is_vector, out, in_):
            if eng_is_vector:
                nc.vector.tensor_copy(out, in_)
            else:
                nc.scalar.copy(out, in_)

        ident = const.tile([P, P], bf16)
        make_identity(nc, ident)
        wz_s = const.tile([CZ, H], bf16)
        nc.gpsimd.dma_start(wz_s, w_z)  # cast f32 -> bf16
        bq_s = const.tile([P, KC], f32)
        nc.sync.dma_start(bq_s, b_q.rearrange("(fo p) -> p fo", p=P))
        mask_s = const.tile([P, JC], f32)
        nc.sync.dma_start(mask_s, mask.rearrange("(jo p) -> p jo", p=P))

        # ---- z: per (j, jc) PE-transpose bias [i,c] block then matmul Wz ----
        # bias loads in natural [i part, (j, c)] layout: 128 contiguous 16KB
        # descriptors per DMA. z_s layout [i_part, jc, j, h]. Emitted in
        # chunks interleaved with the projection stages so PE never idles.
        z_s = big_p.tile([P, JC, P, H], bf16, tag="z")
        if "z" in skip:
            nc.vector.memset(z_s, 0.0)

        def emit_z_chunk(jc, jws=range(4)):
            if "z" in skip:
                return
            for jw in jws:
                bt = bstage_p.tile([P, 32, CZ], bf16, tag="bt", name=f"bt_{jc}_{jw}")
                if "zdma" in skip:
                    nc.vector.memset(bt, 0.001)
                else:
                    for bh in range(2):
                        nc.gpsimd.dma_start(
                            bt[:, bh * 16 : (bh + 1) * 16, :],
                            bias_c[
                                :,
                                jc * P + jw * 32 + bh * 16 : jc * P
                                + jw * 32
                                + (bh + 1) * 16,
                                :,
                            ],
                        )
                zp = bigps.tile([P, 512], f32, tag="big", name=f"zp_{jc}_{jw}")
                for j8 in range(4):  # 8 j per transpose bank
                    tb = tpsum.tile([P, 1024], bf16, tag="tb", name=f"tb_{jc}_{jw}_{j8}")
                    bT = bt_p.tile([P, 8, P], bf16, tag="bT", name=f"bT_{jc}_{jw}_{j8}")
                    for jl in range(8):
                        nc.tensor.transpose(
                            tb[:, jl * P : (jl + 1) * P],
                            bt[:, j8 * 8 + jl, :],
                            ident,
                        )
                    copy_on(j8 % 2 == 0, bT, tb)
                    if "zmm" in skip:
                        if j8 == 0:
                            nc.vector.memset(zp, 0.0)
                            nc.vector.tensor_scalar_mul(zp[:, :1], bT[:, 0, :1], 1.0)
                    else:
                        for jl in range(8):
                            jj = j8 * 8 + jl
                            nc.tensor.matmul(
                                zp[:, jj * H : (jj + 1) * H],
                                bT[:, jl, :],
                                wz_s,
                                start=True,
                                stop=True,
                            )
                nc.vector.tensor_copy(
                    z_s[:, jc, jw * 32 : (jw + 1) * 32, :], zp
                )

        emit_z_chunk(0, range(0, 2))

        # ---- transposed activations: sT [c,i], kinT [c,j] ----
        snat = wnat_p.tile([P, CS], bf16, tag="wnat")
        nc.gpsimd.dma_start(snat, s_c)
        sT = small_p.tile([P, KC, NI], bf16, tag="sT")
        for ch in range(2):
            tb = tpsum.tile([P, 1024], bf16, tag="tb")
            for co in range(ch * 4, ch * 4 + 4):
                nc.tensor.transpose(
                    tb[:, (co % 4) * P : (co % 4 + 1) * P],
                    snat[:, co * P : (co + 1) * P],
                    ident,
                )
            nc.vector.tensor_copy(
                sT[:, ch * 4 : (ch + 1) * 4, :], tb[:, : 4 * P]
            )

        kinT = kin_p.tile([P, KC, J], bf16)
        kr = k_in.rearrange("(jo p) c -> p jo c", p=P)
        for jh in range(2):
            knat = wnat_p.tile([P, 4, CS], bf16, tag="wnat")
            nc.gpsimd.dma_start(knat, kr[:, jh * 4 : (jh + 1) * 4, :])
            for co in range(KC):
                tb = tpsum.tile([P, 1024], bf16, tag="tb")
                for jo in range(4):
                    nc.tensor.transpose(
                        tb[:, jo * P : (jo + 1) * P],
                        knat[:, jo, co * P : (co + 1) * P],
                        ident,
                    )
                copy_on(co % 2 == 0, kinT[:, co, jh * 512 : (jh + 1) * 512], tb[:, :512])

        emit_z_chunk(0, range(2, 4))

        # ---- weights: load natural [f,c], PE-transpose to [c,f] ----
        def load_wT(w_ap, dst_pool, tag):
            wT = dst_pool.tile([P, KC, CS], bf16, tag=tag)
            wr = w_ap.rearrange("(fo p) c -> p fo c", p=P)
            for fh in range(4):
                wnat = wnat_p.tile([P, 2, CS], bf16, tag="wnat")
                nc.gpsimd.dma_start(wnat, wr[:, fh * 2 : (fh + 1) * 2, :])
                for ch in range(2):
                    tb = tpsum.tile([P, 1024], bf16, tag="tb")
                    for co in range(ch * 4, ch * 4 + 4):
                        for fo in range(2):
                            nc.tensor.transpose(
                                tb[:, ((co % 4) * 2 + fo) * P : ((co % 4) * 2 + fo + 1) * P],
                                wnat[:, fo, co * P : (co + 1) * P],
                                ident,
                            )
                    for co in range(ch * 4, ch * 4 + 4):
                        copy_on(
                            co % 2 == 0,
                            wT[:, co, fh * 256 : (fh + 1) * 256],
                            tb[:, (co % 4) * 2 * P : ((co % 4) * 2 + 2) * P],
                        )
            return wT

        # ---- q projection: qT [f,i] = Wq s^T, scaled by 1/sqrt(D), +bq ----
        wqT = load_wT(w_q, wt_p, "wt")
        qT = small_p.tile([P, KC, NI], bf16, tag="qT")
        for fo in range(KC):
            ps = bigps.tile([P, 512], f32, tag="big")
            for co in range(KC):
                nc.tensor.matmul(
                    ps[:, :NI],
                    wqT[:, co, fo * P : (fo + 1) * P],
                    sT[:, co, :],
                    start=(co == 0),
                    stop=(co == KC - 1),
                )
            nc.vector.tensor_scalar(
                qT[:, fo, :],
                ps[:, :NI],
                bq_s[:, fo : fo + 1],
                1.0 / np.sqrt(D),
                ALU.add,
                ALU.mult,
            )

        emit_z_chunk(1)

        # ---- k projection: kT [f,j] = Wk k_in^T ----
        wkT = load_wT(w_k, wt_p, "wt")
        kT = big_p.tile([P, KC, J], bf16, tag="kT")
        for fo in range(KC):
            for jh in range(2):
                ps = bigps.tile([P, 512], f32, tag="big")
                for co in range(KC):
                    nc.tensor.matmul(
                        ps,
                        wkT[:, co, fo * P : (fo + 1) * P],
                        kinT[:, co, jh * 512 : (jh + 1) * 512],
                        start=(co == 0),
                        stop=(co == KC - 1),
                    )
                copy_on(jh == 0, kT[:, fo, jh * 512 : (jh + 1) * 512], ps)

        emit_z_chunk(2)

        # ---- v projection: v [j, h, d|mask] = k_in Wv^T, masked ----
        wvT = load_wT(w_v, wt_p, "wt")
        v_s = big_p.tile([P, JC, H, D + 1], bf16, tag="v")
        for jo in range(JC):
            for fh in range(2):
                ps = bigps.tile([P, 512], f32, tag="big")
                for co in range(KC):
                    nc.tensor.matmul(
                        ps,
                        kinT[:, co, jo * P : (jo + 1) * P],
                        wvT[:, co, fh * 512 : (fh + 1) * 512],
                        start=(co == 0),
                        stop=(co == KC - 1),
                    )
                nc.vector.tensor_scalar_mul(
                    v_s[:, jo, fh * 8 : (fh + 1) * 8, 0:D],
                    ps,
                    mask_s[:, jo : jo + 1],
                )
            nc.vector.tensor_copy(
                v_s[:, jo, :, D : D + 1],
                mask_s[:, jo : jo + 1, None].to_broadcast((P, H, 1)),
            )

        emit_z_chunk(3)

        # ---- g projection: g [i, f] = sigmoid(s Wg^T) ----
        wgT = load_wT(w_g, wt_p, "wt")
        g_s = small_p.tile([P, CS], bf16, tag="g")
        for fh in range(2):
            ps = bigps.tile([P, 512], f32, tag="big")
            for co in range(KC):
                nc.tensor.matmul(
                    ps,
                    sT[:, co, :],
                    wgT[:, co, fh * 512 : (fh + 1) * 512],
                    start=(co == 0),
                    stop=(co == KC - 1),
                )
            nc.scalar.activation(g_s[:, fh * 512 : (fh + 1) * 512], ps, AF.Sigmoid)

        emit_z_chunk(4)
        woT = load_wT(w_o, wot_p, "wot")
        emit_z_chunk(5)

        # ---- attention (scores-major-i), two passes interleaved with z ----
        o_s = small_p.tile([P, CS], bf16, tag="o")
        o_acc = small_p.tile([P, H, D + 1], f32, tag="oacc")
        if "attn" in skip:
            nc.vector.memset(o_s, 0.0)

        def emit_attn_pass(jh):
            if "attn" in skip:
                return
            for h in range(H):
                fo, pb = h // 2, (h % 2) * D
                qk = qkps.tile([P, 512], f32, tag="qk", name=f"qk_{jh}_{h}")
                nc.tensor.matmul(
                    qk,
                    qT[pb : pb + D, fo, :],
                    kT[pb : pb + D, fo, jh * 512 : (jh + 1) * 512],
                    start=True,
                    stop=True,
                )
                st = st_p.tile([P, 512], f32, tag="st", name=f"st_{jh}_{h}")
                nc.vector.tensor_tensor(
                    st,
                    qk,
                    z_s[:, 4 * jh : 4 * (jh + 1), :, h].rearrange("p a b -> p (a b)"),
                    ALU.add,
                )
                et = et_p.tile([P, 512], bf16, tag="et", name=f"et_{jh}_{h}")
                nc.scalar.activation(et, st, AF.Exp)
                tb = tpsum.tile([P, 1024], bf16, tag="tb", name=f"etb_{jh}_{h}")
                for jl in range(4):
                    nc.tensor.transpose(
                        tb[:, jl * P : (jl + 1) * P],
                        et[:, jl * P : (jl + 1) * P],
                        ident,
                    )
                etT = et_p.tile([P, 4, P], bf16, tag="etT", name=f"etT_{jh}_{h}")
                copy_on(h % 2 == 0, etT, tb[:, : 4 * P])
                op = ops.tile([P, 512], f32, tag="op", name=f"op_{jh}_{h}")
                for jc4 in range(4):
                    nc.tensor.matmul(
                        op[:, : D + 1],
                        etT[:, jc4, :],
                        v_s[:, jh * 4 + jc4, h, :],
                        start=(jc4 == 0),
                        stop=(jc4 == 3),
                    )
                if jh == 0:
                    nc.vector.tensor_copy(o_acc[:, h, :], op[:, : D + 1])
                else:
                    nc.vector.tensor_tensor(
                        o_acc[:, h, :], op[:, : D + 1], o_acc[:, h, :], ALU.add
                    )

        def emit_attn_final():
            if "attn" in skip:
                return
            for h in range(H):
                rec = r_p.tile([P, 1], f32, tag="r", name=f"rec_{h}")
                nc.vector.reciprocal(rec, o_acc[:, h, D : D + 1])
                nc.vector.tensor_scalar_mul(
                    o_s[:, h * D : (h + 1) * D], o_acc[:, h, 0:D], rec
                )

        emit_attn_pass(0)
        for _jc in (6, 7):
            emit_z_chunk(_jc)
        emit_attn_pass(1)
        emit_attn_final()

        # ---- gating + output projection ----
        nc.vector.tensor_mul(g_s, g_s, o_s)
        goT = small_p.tile([P, KC, NI], bf16, tag="goT")
        for gh in range(2):
            tb = tpsum.tile([P, 1024], bf16, tag="tb")
            for fo in range(gh * 4, gh * 4 + 4):
                nc.tensor.transpose(
                    tb[:, (fo % 4) * P : (fo % 4 + 1) * P],
                    g_s[:, fo * P : (fo + 1) * P],
                    ident,
                )
            nc.vector.tensor_copy(goT[:, gh * 4 : (gh + 1) * 4, :], tb[:, : 4 * P])

        for fh in range(2):
            ps = bigps.tile([P, 512], f32, tag="big")
            for fo in range(KC):
                nc.tensor.matmul(
                    ps,
                    goT[:, fo, :],
                    woT[:, fo, fh * 512 : (fh + 1) * 512],
                    start=(fo == 0),
                    stop=(fo == KC - 1),
                )
            out_s = outs_p.tile([P, 512], f32, tag="outs", name=f"out_s{fh}")
            nc.vector.tensor_copy(out_s, ps)
            nc.sync.dma_start(out_d[:, fh * 512 : (fh + 1) * 512], out_s)

    nc.compile()
    return nc


def kernel(**inputs):
    global _last_results
    from concourse.bass_utils import run_bass_kernel_spmd

    s = np.ascontiguousarray(np.asarray(inputs["s"], dtype=np.float32)[0])
    k_in = np.ascontiguousarray(np.asarray(inputs["k_in"], dtype=np.float32)[0])
    mask = np.ascontiguousarray(np.asarray(inputs["mask"], dtype=np.float32)[0])
    bias = np.asarray(inputs["bias"], dtype=np.float32)[0]
    wq = np.ascontiguousarray(np.asarray(inputs["Wq"], dtype=np.float32))
    wk = np.ascontiguousarray(np.asarray(inputs["Wk"], dtype=np.float32))
    wv = np.ascontiguousarray(np.asarray(inputs["Wv"], dtype=np.float32))
    wg = np.ascontiguousarray(np.asarray(inputs["Wg"], dtype=np.float32))
    wo = np.ascontiguousarray(np.asarray(inputs["Wo"], dtype=np.float32))
    bq = np.ascontiguousarray(np.asarray(inputs["bq"], dtype=np.float32))
    wz = np.ascontiguousarray(np.asarray(inputs["Wz"], dtype=np.float32))
    mult = int(np.asarray(inputs.get("multiplicity", 1)))
    assert mult == 1, f"multiplicity={mult} not supported (B=1)"

    nc = _build_program()

    in_maps = []
    for c in range(NCORES):
        in_maps.append(
            {
                "s_c": np.ascontiguousarray(s[c * NI : (c + 1) * NI]),
                "bias_c": np.ascontiguousarray(bias[c * NI : (c + 1) * NI]),
                "k_in": k_in,
                "mask": mask,
                "w_q": wq,
                "w_k": wk,
                "w_v": wv,
                "w_g": wg,
                "w_o": wo,
                "b_q": bq,
                "w_z": wz,
            }
        )

    try:
        res = run_bass_kernel_spmd(nc, in_maps, core_ids=list(range(NCORES)))
    except Exception:
        # transient device-unrecoverable errors have been observed on a
        # first attempt; one retry has always succeeded
        import time as _time

        _time.sleep(5.0)
        res = run_bass_kernel_spmd(nc, in_maps, core_ids=list(range(NCORES)))
    _last_results = res
    out = np.concatenate([r["out"] for r in res.results], axis=0)
    return out.reshape(B, I, CS).astype(np.float32)


if __name__ == "__main__":
    rng = np.random.default_rng(0)
    ins = {
        "s": rng.standard_normal((B, I, CS), dtype=np.float32),
        "k_in": rng.standard_normal((B, J, CS), dtype=np.float32),
        "mask": np.ones((B, J), np.float32),
        "bias": rng.standard_normal((B, I, J, CZ), dtype=np.float32),
        "Wq": rng.standard_normal((CS, CS), dtype=np.float32) * 0.02,
        "bq": rng.standard_normal((CS,), dtype=np.float32) * 0.02,
        "Wk": rng.standard_normal((CS, CS), dtype=np.float32) * 0.02,
        "Wv": rng.standard_normal((CS, CS), dtype=np.float32) * 0.02,
        "Wg": rng.standard_normal((CS, CS), dtype=np.float32) * 0.02,
        "Wo": rng.standard_normal((CS, CS), dtype=np.float32) * 0.02,
        "Wz": rng.standard_normal((CZ, H), dtype=np.float32) * 0.02,
        "multiplicity": 1,
    }
    out = kernel(**ins)
    print(out.shape, out.dtype)



# revision 2
# speedup vs baseline: 1.0444x; 1.0444x over previous
# Trainium2 Bass kernel for nn_CrossAttention (B=1, I=J=1024, C_S=1024,
# C_Z=128, H=16, D=64), sharded over the query dim i across 8 NeuronCores.
#
# v2: all large inputs are pre-cast to bf16 and pre-transposed on the host,
# halving HBM traffic and eliminating nearly all PE transposes.
#   - weights arrive as W^T [in, out] bf16; s, k_in arrive transposed
#     [c, i] / [c, j] bf16 so projections need no on-chip transposes.
#   - bias [i, j, c] bf16 is loaded via XBAR DMA-transpose into [c, (i j)]
#     chunks; z[j, i, h] = bias^T Wz comes from per-i stationary matmuls.
#   - scores are computed j-major: qk [j, i] accumulates on top of a
#     DVE-prefilled z slice in PSUM, exp feeds the o-matmul directly
#     (no transposes in the attention inner loop). Softmax denominator
#     comes from an extra masked ones-column on v.
#
# kernel(**inputs) takes FULL inputs, shards on host, runs SPMD on cores 0-7,
# gathers to the full [1, 1024, 1024] output.

import numpy as np

B, I, J, CS, CZ, H, D = 1, 1024, 1024, 1024, 128, 16, 64
NCORES = 8
NI = I // NCORES  # 128 query rows per core
P = 128
KC = CS // P  # 8 contraction chunks
JC = J // P  # 8 key chunks
IC8 = 8  # i rows per bias chunk
NCHUNK = NI // IC8  # 16 bias chunks

_last_results = None


def _build_program():
    from contextlib import ExitStack

    import concourse.mybir as mybir
    import concourse.tile as tile
    from concourse import bacc
    from concourse.masks import make_identity

    f32 = mybir.dt.float32
    bf16 = mybir.dt.bfloat16
    AF = mybir.ActivationFunctionType
    ALU = mybir.AluOpType

    nc = bacc.Bacc("TRN2", target_bir_lowering=False, debug=False)

    # ---- dram io (transposed/bf16 layouts prepared on host) ----
    sT_d = nc.dram_tensor("sT", [CS, NI], bf16, kind="ExternalInput").ap()
    kinT_d = nc.dram_tensor("kinT", [CS, J], bf16, kind="ExternalInput").ap()
    bias_d = nc.dram_tensor("bias_c", [NI * J, CZ], bf16, kind="ExternalInput").ap()
    wqT_d = nc.dram_tensor("wqT", [CS, CS], bf16, kind="ExternalInput").ap()
    wkT_d = nc.dram_tensor("wkT", [CS, CS], bf16, kind="ExternalInput").ap()
    wvT_d = nc.dram_tensor("wvT", [CS, CS], bf16, kind="ExternalInput").ap()
    wgT_d = nc.dram_tensor("wgT", [CS, CS], bf16, kind="ExternalInput").ap()
    woT_d = nc.dram_tensor("woT", [CS, CS], bf16, kind="ExternalInput").ap()
    wz_d = nc.dram_tensor("w_z", [CZ, H], bf16, kind="ExternalInput").ap()
    bq_d = nc.dram_tensor("b_q", [CS], f32, kind="ExternalInput").ap()
    mask_d = nc.dram_tensor("mask", [J], f32, kind="ExternalInput").ap()
    out_d = nc.dram_tensor("out", [NI, CS], f32, kind="ExternalOutput").ap()

    with tile.TileContext(nc) as tc, ExitStack() as ctx:
        pool = lambda name, bufs: ctx.enter_context(tc.tile_pool(name=name, bufs=bufs))
        ppool = lambda name, bufs: ctx.enter_context(
            tc.tile_pool(name=name, bufs=bufs, space="PSUM")
        )

        const = pool("const", 1)
        act_p = pool("act", 1)  # persistent small activations
        big_p = pool("big", 1)  # persistent big tensors (kinT, kT, v, z)
        bstage_p = pool("bstage", 3)  # bias^T chunks
        wstage_p = pool("wstage", 2)  # weight chunks
        et_p = pool("et", 3)
        outs_p = pool("outs", 2)

        big_ps = ppool("bigps", 2)  # [128,512] f32: projections + o-proj
        z_ps = ppool("zps", 2)  # [128,512] f32: z accumulation
        qk_ps = ppool("qkps", 2)  # [128,128] f32: attention scores
        op_ps = ppool("ops", 1)  # [128,65] f32: o accumulator
        tp_ps = ppool("tps", 1)  # transpose target for g*o

        def copy_on(eng_is_vector, out, in_):
            if eng_is_vector:
                nc.vector.tensor_copy(out, in_)
            else:
                nc.scalar.copy(out, in_)

        # ---- constants / small loads (sync ring) ----
        ident = const.tile([P, P], bf16)
        make_identity(nc, ident)
        wz_s = const.tile([CZ, H], bf16)
        nc.sync.dma_start(wz_s, wz_d)
        bq_s = const.tile([P, KC], f32)
        nc.sync.dma_start(bq_s, bq_d.rearrange("(fo p) -> p fo", p=P))
        mask_s = const.tile([P, JC], f32)
        nc.sync.dma_start(mask_s, mask_d.rearrange("(jo p) -> p jo", p=P))
        sT_s = act_p.tile([P, KC, NI], bf16, tag="sT")
        nc.sync.dma_start(sT_s, sT_d.rearrange("(co p) i -> p co i", p=P))

        # ---- big activation loads (scalar ring) ----
        kinT_s = big_p.tile([P, KC, J], bf16, tag="kinT")
        nc.scalar.dma_start(kinT_s, kinT_d.rearrange("(co p) j -> p co j", p=P))

        def load_w(w_ap, tag):
            w = wstage_p.tile([P, KC, CS], bf16, tag="w", name=tag)
            nc.scalar.dma_start(w, w_ap.rearrange("(co p) f -> p co f", p=P))
            return w

        # ---- z: bias^T chunks via XBAR DMA transpose + per-i matmuls ----
        # z_s layout: [j_part, jc, i, h] (bf16)
        z_s = big_p.tile([P, JC, NI, H], bf16, tag="z")

        def z_chunk(ic):
            # bias rows [ic*8*J, (ic+1)*8*J) -> biasT [c=128, 8 i, 1024 j]
            bt = bstage_p.tile([P, IC8, J], bf16, tag="bt", name=f"bt_{ic}")
            nc.sync.dma_start(
                bt, bias_d[ic * IC8 * J : (ic + 1) * IC8 * J, :], transpose=True
            )
            # 4 jc per psum bank: [j=128, (4 jc, 8 i, 16 h)=512]
            for jh in range(2):
                zp = z_ps.tile([P, 512], f32, tag="zp", name=f"zp_{ic}_{jh}")
                for jcl in range(4):
                    jc = jh * 4 + jcl
                    for il in range(IC8):
                        nc.tensor.matmul(
                            zp[:, jcl * 128 + il * H : jcl * 128 + (il + 1) * H],
                            bt[:, il, jc * P : (jc + 1) * P],
                            wz_s,
                            start=True,
                            stop=True,
                        )
                nc.vector.tensor_copy(
                    z_s[:, jh * 4 : (jh + 1) * 4, ic * IC8 : (ic + 1) * IC8, :],
                    zp.rearrange("p (a b c) -> p a b c", a=4, b=IC8),
                )

        # ---- q projection: qT [f, i] = Wq s^T (+bq, /sqrt(D)) ----
        wq_s = load_w(wqT_d, "wq")
        qT_s = act_p.tile([P, KC, NI], bf16, tag="qT")

        def q_proj():
            for fh in range(2):
                ps = big_ps.tile([P, 512], f32, tag="big", name=f"qp_{fh}")
                for fol in range(4):
                    fo = fh * 4 + fol
                    for co in range(KC):
                        nc.tensor.matmul(
                            ps[:, fol * P : (fol + 1) * P],
                            wq_s[:, co, fo * P : (fo + 1) * P],
                            sT_s[:, co, :],
                            start=(co == 0),
                            stop=(co == KC - 1),
                        )
                for fol in range(4):
                    fo = fh * 4 + fol
                    nc.vector.tensor_scalar(
                        qT_s[:, fo, :],
                        ps[:, fol * P : (fol + 1) * P],
                        bq_s[:, fo : fo + 1],
                        1.0 / np.sqrt(D),
                        ALU.add,
                        ALU.mult,
                    )

        q_proj()
        z_chunk(0)
        z_chunk(1)

        # ---- k projection: kT [f, j] = Wk k_in^T ----
        wk_s = load_w(wkT_d, "wk")
        kT_s = big_p.tile([P, KC, J], bf16, tag="kT")
        for fo in range(KC):
            for jh in range(2):
                ps = big_ps.tile([P, 512], f32, tag="big", name=f"kp_{fo}_{jh}")
                for co in range(KC):
                    nc.tensor.matmul(
                        ps,
                        wk_s[:, co, fo * P : (fo + 1) * P],
                        kinT_s[:, co, jh * 512 : (jh + 1) * 512],
                        start=(co == 0),
                        stop=(co == KC - 1),
                    )
                copy_on(jh == 0, kT_s[:, fo, jh * 512 : (jh + 1) * 512], ps)
            if fo % 2 == 1:
                z_chunk(2 + fo // 2)

        # ---- v projection: v [j, h, d|mask] = k_in Wv^T, masked ----
        wv_s = load_w(wvT_d, "wv")
        v_s = big_p.tile([P, JC, H, D + 1], bf16, tag="v")
        for jo in range(JC):
            for fh in range(2):
                ps = big_ps.tile([P, 512], f32, tag="big", name=f"vp_{jo}_{fh}")
                for co in range(KC):
                    nc.tensor.matmul(
                        ps,
                        kinT_s[:, co, jo * P : (jo + 1) * P],
                        wv_s[:, co, fh * 512 : (fh + 1) * 512],
                        start=(co == 0),
                        stop=(co == KC - 1),
                    )
                nc.vector.tensor_scalar_mul(
                    v_s[:, jo, fh * 8 : (fh + 1) * 8, 0:D],
                    ps,
                    mask_s[:, jo : jo + 1],
                )
            nc.vector.tensor_copy(
                v_s[:, jo, :, D : D + 1],
                mask_s[:, jo : jo + 1, None].to_broadcast((P, H, 1)),
            )
            if jo % 2 == 1:
                z_chunk(6 + jo // 2)

        # ---- g projection: g [i, f] = sigmoid(s Wg^T) ----
        wg_s = load_w(wgT_d, "wg")
        g_s = act_p.tile([P, CS], bf16, tag="g")
        for fh in range(2):
            ps = big_ps.tile([P, 512], f32, tag="big", name=f"gp_{fh}")
            for co in range(KC):
                nc.tensor.matmul(
                    ps,
                    sT_s[:, co, :],
                    wg_s[:, co, fh * 512 : (fh + 1) * 512],
                    start=(co == 0),
                    stop=(co == KC - 1),
                )
            nc.scalar.activation(g_s[:, fh * 512 : (fh + 1) * 512], ps, AF.Sigmoid)
            z_chunk(10 + fh)

        wo_s = load_w(woT_d, "wo")
        for ic in range(12, NCHUNK):
            z_chunk(ic)

        # ---- attention: per head, j-major scores on top of prefilled z ----
        o_s = act_p.tile([P, CS], bf16, tag="o")
        for h in range(H):
            fo, pb = h // 2, (h % 2) * D
            op = op_ps.tile([P, D + 1], f32, tag="op", name=f"op_{h}")
            for jc in range(JC):
                qk = qk_ps.tile([P, P], f32, tag="qk", name=f"qk_{h}_{jc}")
                # prefill z slice [j, i] for (h, jc), then accumulate qk
                nc.vector.tensor_copy(qk, z_s[:, jc, :, h])
                nc.tensor.matmul(
                    qk,
                    kT_s[pb : pb + D, fo, jc * P : (jc + 1) * P],
                    qT_s[pb : pb + D, fo, :],
                    start=False,
                    stop=True,
                )
                et = et_p.tile([P, P], bf16, tag="et", name=f"et_{h}_{jc}")
                nc.scalar.activation(et, qk, AF.Exp)
                nc.tensor.matmul(
                    op,
                    et,
                    v_s[:, jc, h, :],
                    start=(jc == 0),
                    stop=(jc == JC - 1),
                )
            rec = et_p.tile([P, 1], f32, tag="rec", name=f"rec_{h}")
            nc.vector.reciprocal(rec, op[:, D : D + 1])
            nc.vector.tensor_scalar_mul(o_s[:, h * D : (h + 1) * D], op[:, 0:D], rec)

        # ---- gating + output projection ----
        nc.vector.tensor_mul(g_s, g_s, o_s)
        goT = act_p.tile([P, KC, NI], bf16, tag="goT")
        for gh in range(2):
            tb = tp_ps.tile([P, 4 * P], bf16, tag="tb", name=f"tb_{gh}")
            for fo in range(gh * 4, gh * 4 + 4):
                nc.tensor.transpose(
                    tb[:, (fo % 4) * P : (fo % 4 + 1) * P],
                    g_s[:, fo * P : (fo + 1) * P],
                    ident,
                )
            nc.vector.tensor_copy(goT[:, gh * 4 : (gh + 1) * 4, :], tb)

        for fh in range(2):
            ps = big_ps.tile([P, 512], f32, tag="big", name=f"op_ps_{fh}")
            for fo in range(KC):
                nc.tensor.matmul(
                    ps,
                    goT[:, fo, :],
                    wo_s[:, fo, fh * 512 : (fh + 1) * 512],
                    start=(fo == 0),
                    stop=(fo == KC - 1),
                )
            out_s = outs_p.tile([P, 512], f32, tag="outs", name=f"out_s{fh}")
            nc.vector.tensor_copy(out_s, ps)
            nc.sync.dma_start(out_d[:, fh * 512 : (fh + 1) * 512], out_s)

    nc.compile()
    return nc


def kernel(**inputs):
    global _last_results
    import ml_dtypes
    from concourse.bass_utils import run_bass_kernel_spmd

    bf = ml_dtypes.bfloat16
    s = np.asarray(inputs["s"], dtype=np.float32)[0]
    k_in = np.asarray(inputs["k_in"], dtype=np.float32)[0]
    mask = np.ascontiguousarray(np.asarray(inputs["mask"], dtype=np.float32)[0])
    bias = np.asarray(inputs["bias"], dtype=np.float32)[0]
    bq = np.ascontiguousarray(np.asarray(inputs["bq"], dtype=np.float32))
    mult = int(np.asarray(inputs.get("multiplicity", 1)))
    assert mult == 1, f"multiplicity={mult} not supported (B=1)"

    # host-side transposes + bf16 casts (cheap vs device HBM savings)
    sT = np.ascontiguousarray(s.T.astype(bf))  # [c, i_full]
    kinT = np.ascontiguousarray(k_in.T.astype(bf))  # [c, j]
    wT = {
        k: np.ascontiguousarray(np.asarray(inputs[k], np.float32).T.astype(bf))
        for k in ("Wq", "Wk", "Wv", "Wg", "Wo")
    }
    wz = np.ascontiguousarray(np.asarray(inputs["Wz"], np.float32).astype(bf))

    nc = _build_program()

    in_maps = []
    for c in range(NCORES):
        bias_c = np.ascontiguousarray(
            bias[c * NI : (c + 1) * NI].reshape(NI * J, CZ).astype(bf)
        )
        in_maps.append(
            {
                "sT": np.ascontiguousarray(sT[:, c * NI : (c + 1) * NI]),
                "kinT": kinT,
                "bias_c": bias_c,
                "wqT": wT["Wq"],
                "wkT": wT["Wk"],
                "wvT": wT["Wv"],
                "wgT": wT["Wg"],
                "woT": wT["Wo"],
                "w_z": wz,
                "b_q": bq,
                "mask": mask,
            }
        )

    try:
        res = run_bass_kernel_spmd(nc, in_maps, core_ids=list(range(NCORES)))
    except Exception:
        # transient device-unrecoverable errors have been observed on a
        # first attempt; one retry has always succeeded
        import time as _time

        _time.sleep(5.0)
        res = run_bass_kernel_spmd(nc, in_maps, core_ids=list(range(NCORES)))
    _last_results = res
    out = np.concatenate([r["out"] for r in res.results], axis=0)
    return out.reshape(B, I, CS).astype(np.float32)


if __name__ == "__main__":
    rng = np.random.default_rng(0)
    ins = {
        "s": rng.standard_normal((B, I, CS), dtype=np.float32),
        "k_in": rng.standard_normal((B, J, CS), dtype=np.float32),
        "mask": np.ones((B, J), np.float32),
        "bias": rng.standard_normal((B, I, J, CZ), dtype=np.float32),
        "Wq": rng.standard_normal((CS, CS), dtype=np.float32) * 0.02,
        "bq": rng.standard_normal((CS,), dtype=np.float32) * 0.02,
        "Wk": rng.standard_normal((CS, CS), dtype=np.float32) * 0.02,
        "Wv": rng.standard_normal((CS, CS), dtype=np.float32) * 0.02,
        "Wg": rng.standard_normal((CS, CS), dtype=np.float32) * 0.02,
        "Wo": rng.standard_normal((CS, CS), dtype=np.float32) * 0.02,
        "Wz": rng.standard_normal((CZ, H), dtype=np.float32) * 0.02,
        "multiplicity": 1,
    }
    out = kernel(**ins)
    print(out.shape, out.dtype)


# revision 9
# speedup vs baseline: 1.8929x; 1.8124x over previous
# Trainium2 Bass kernel for nn_CrossAttention (B=1, I=J=1024, C_S=1024,
# C_Z=128, H=16, D=64), sharded over the query dim i across 8 NeuronCores.
#
# v4: all large inputs are pre-cast (bias to fp8-e4m3, rest to bf16) AND
# pre-laid-out on the host so every device DMA is a plain contiguous
# [128, N] load (128 big descriptors, full HBM rate, no on-chip transposes):
#   - weights arrive as W^T in [p, co, f] chunk layout; s, k_in transposed
#     likewise; bias arrives as bias^T [c, i, j] per core in e4m3 (the z
#     pair-bias projection tolerates fp8: measured out-relerr 0.006 vs the
#     f64 reference, on top of ~0.005 from the bf16 pipeline).
#   - z[j, i, h] = bias^T Wz via per-(i, jc) stationary matmuls (N=16,
#     fp8 stationary x bf16 moving), interleaved with the projections.
#   - attention is j-major: scores [j, i] accumulate on a DVE-prefilled
#     z slice in PSUM (4 key-chunks per 512-wide bank), one exp per bank,
#     exp output feeds the o-matmul directly as the stationary operand.
#     Softmax denominator comes from an extra masked ones-column on v.
#
# kernel(**inputs) takes FULL inputs, shards on host, runs SPMD on cores 0-7,
# gathers to the full [1, 1024, 1024] output.

import numpy as np

B, I, J, CS, CZ, H, D = 1, 1024, 1024, 1024, 128, 16, 64
NCORES = 8
NI = I // NCORES  # 128 query rows per core
P = 128
KC = CS // P  # 8 contraction chunks
JC = J // P  # 8 key chunks
IC8 = 8  # i rows per bias chunk
NCHUNK = NI // IC8  # 16 bias chunks

_last_results = None


def _build_program():
    from contextlib import ExitStack

    import concourse.mybir as mybir
    import concourse.tile as tile
    from concourse import bacc
    from concourse.masks import make_identity

    f32 = mybir.dt.float32
    bf16 = mybir.dt.bfloat16
    fp8 = mybir.dt.float8e4
    AF = mybir.ActivationFunctionType
    ALU = mybir.AluOpType

    nc = bacc.Bacc("TRN2", target_bir_lowering=False, debug=False)

    # ---- dram io (host-prepared layouts, all partition-major) ----
    sT_d = nc.dram_tensor("sT", [P, KC, NI], bf16, kind="ExternalInput").ap()
    kinT_d = nc.dram_tensor("kinT", [P, KC, J], bf16, kind="ExternalInput").ap()
    biasT_d = nc.dram_tensor("biasT", [P, NI, J], fp8, kind="ExternalInput").ap()
    wqT_d = nc.dram_tensor("wqT", [P, KC, CS], bf16, kind="ExternalInput").ap()
    wkT_d = nc.dram_tensor("wkT", [P, KC, CS], bf16, kind="ExternalInput").ap()
    wvT_d = nc.dram_tensor("wvT", [P, KC, CS], bf16, kind="ExternalInput").ap()
    wgT_d = nc.dram_tensor("wgT", [P, KC, CS], bf16, kind="ExternalInput").ap()
    woT_d = nc.dram_tensor("woT", [P, KC, CS], bf16, kind="ExternalInput").ap()
    wz_d = nc.dram_tensor("w_z", [CZ, H], bf16, kind="ExternalInput").ap()
    bq_d = nc.dram_tensor("b_q", [P, KC], f32, kind="ExternalInput").ap()
    mask_d = nc.dram_tensor("mask", [P, JC], f32, kind="ExternalInput").ap()
    out_d = nc.dram_tensor("out", [NI, CS], f32, kind="ExternalOutput").ap()

    with tile.TileContext(nc) as tc, ExitStack() as ctx:
        pool = lambda name, bufs: ctx.enter_context(tc.tile_pool(name=name, bufs=bufs))
        ppool = lambda name, bufs: ctx.enter_context(
            tc.tile_pool(name=name, bufs=bufs, space="PSUM")
        )

        const = pool("const", 1)
        act_p = pool("act", 1)  # persistent small activations
        big_p = pool("big", 1)  # persistent big tensors (kinT, kT, v, z)
        bstage_p = pool("bstage", 3)  # bias^T chunks
        wstage_p = pool("wstage", 2)  # weight chunks
        et_p = pool("et", 3)
        st_p = pool("st", 2)
        outs_p = pool("outs", 2)

        big_ps = ppool("bigps", 2)  # [128,512] f32: projections / o-proj / go-T
        zq_ps = ppool("zqps", 3)  # [128,512] f32: z accumulation, then qk banks
        op_ps = ppool("ops", 2)  # [128,65] f32: o accumulators

        def copy_on(eng_is_vector, out, in_):
            if eng_is_vector:
                nc.vector.tensor_copy(out, in_)
            else:
                nc.scalar.copy(out, in_)

        # ---- constants / small loads (sync ring) ----
        ident = const.tile([P, P], bf16)
        make_identity(nc, ident)
        wz_s = const.tile([CZ, H], bf16)
        nc.sync.dma_start(wz_s, wz_d)
        bq_s = const.tile([P, KC], f32)
        nc.sync.dma_start(bq_s, bq_d)
        mask_s = const.tile([P, JC], f32)
        nc.sync.dma_start(mask_s, mask_d)
        sT_s = act_p.tile([P, KC, NI], bf16, tag="sT")
        nc.sync.dma_start(sT_s, sT_d)

        # ---- big activation loads (scalar ring) ----
        kinT_s = big_p.tile([P, KC, J], bf16, tag="kinT")
        nc.scalar.dma_start(kinT_s, kinT_d)

        def load_w(w_ap, tag):
            w = wstage_p.tile([P, KC, CS], bf16, tag="w", name=tag)
            nc.scalar.dma_start(w, w_ap)
            return w

        # ---- z: bias^T chunks (plain DMA) + per-(i, jc) matmuls ----
        # z_s layout: [j_part, jc, i, h] (bf16)
        z_s = big_p.tile([P, JC, NI, H], bf16, tag="z")

        def z_chunk(ic):
            bt = bstage_p.tile([P, IC8, J], fp8, tag="bt", name=f"bt_{ic}")
            nc.sync.dma_start(bt, biasT_d[:, ic * IC8 : (ic + 1) * IC8, :])
            # 4 jc per psum bank: [j=128, (4 jc, 8 i, 16 h)=512]
            for jh in range(2):
                zp = zq_ps.tile([P, 512], f32, tag="zq", name=f"zp_{ic}_{jh}")
                for jcl in range(4):
                    jc = jh * 4 + jcl
                    for il in range(IC8):
                        nc.tensor.matmul(
                            zp[:, jcl * 128 + il * H : jcl * 128 + (il + 1) * H],
                            bt[:, il, jc * P : (jc + 1) * P],
                            wz_s,
                            start=True,
                            stop=True,
                        )
                nc.vector.tensor_copy(
                    z_s[:, jh * 4 : (jh + 1) * 4, ic * IC8 : (ic + 1) * IC8, :],
                    zp.rearrange("p (a b c) -> p a b c", a=4, b=IC8),
                )

        # ---- q projection: qT [f, i] = Wq s^T (+bq, /sqrt(D)) ----
        wq_s = load_w(wqT_d, "wq")
        qT_s = act_p.tile([P, KC, NI], bf16, tag="qT")

        def q_proj():
            for fh in range(2):
                ps = big_ps.tile([P, 512], f32, tag="big", name=f"qp_{fh}")
                for fol in range(4):
                    fo = fh * 4 + fol
                    for co in range(KC):
                        nc.tensor.matmul(
                            ps[:, fol * P : (fol + 1) * P],
                            wq_s[:, co, fo * P : (fo + 1) * P],
                            sT_s[:, co, :],
                            start=(co == 0),
                            stop=(co == KC - 1),
                        )
                for fol in range(4):
                    fo = fh * 4 + fol
                    nc.vector.tensor_scalar(
                        qT_s[:, fo, :],
                        ps[:, fol * P : (fol + 1) * P],
                        bq_s[:, fo : fo + 1],
                        1.0 / np.sqrt(D),
                        ALU.add,
                        ALU.mult,
                    )

        q_proj()
        z_chunk(0)
        z_chunk(1)

        # ---- k projection: kT [f, j] = Wk k_in^T ----
        wk_s = load_w(wkT_d, "wk")
        kT_s = big_p.tile([P, KC, J], bf16, tag="kT")
        for fo in range(KC):
            for jh in range(2):
                ps = big_ps.tile([P, 512], f32, tag="big", name=f"kp_{fo}_{jh}")
                for co in range(KC):
                    nc.tensor.matmul(
                        ps,
                        wk_s[:, co, fo * P : (fo + 1) * P],
                        kinT_s[:, co, jh * 512 : (jh + 1) * 512],
                        start=(co == 0),
                        stop=(co == KC - 1),
                    )
                copy_on(jh == 0, kT_s[:, fo, jh * 512 : (jh + 1) * 512], ps)
            if fo % 2 == 1:
                z_chunk(2 + fo // 2)

        # ---- v projection: v [j, h, d|mask] = k_in Wv^T, masked ----
        wv_s = load_w(wvT_d, "wv")
        v_s = big_p.tile([P, JC, H, D + 1], bf16, tag="v")
        for jo in range(JC):
            for fh in range(2):
                ps = big_ps.tile([P, 512], f32, tag="big", name=f"vp_{jo}_{fh}")
                for co in range(KC):
                    nc.tensor.matmul(
                        ps,
                        kinT_s[:, co, jo * P : (jo + 1) * P],
                        wv_s[:, co, fh * 512 : (fh + 1) * 512],
                        start=(co == 0),
                        stop=(co == KC - 1),
                    )
                nc.vector.tensor_scalar_mul(
                    v_s[:, jo, fh * 8 : (fh + 1) * 8, 0:D],
                    ps,
                    mask_s[:, jo : jo + 1],
                )
            nc.vector.tensor_copy(
                v_s[:, jo, :, D : D + 1],
                mask_s[:, jo : jo + 1, None].to_broadcast((P, H, 1)),
            )
            if jo % 2 == 1:
                z_chunk(6 + jo // 2)

        # ---- g projection: g [i, f] = sigmoid(s Wg^T) ----
        wg_s = load_w(wgT_d, "wg")
        g_s = act_p.tile([P, CS], bf16, tag="g")
        for fh in range(2):
            ps = big_ps.tile([P, 512], f32, tag="big", name=f"gp_{fh}")
            for co in range(KC):
                nc.tensor.matmul(
                    ps,
                    sT_s[:, co, :],
                    wg_s[:, co, fh * 512 : (fh + 1) * 512],
                    start=(co == 0),
                    stop=(co == KC - 1),
                )
            nc.scalar.activation(g_s[:, fh * 512 : (fh + 1) * 512], ps, AF.Sigmoid)
            z_chunk(10 + fh)

        wo_s = load_w(woT_d, "wo")
        for ic in range(12, NCHUNK):
            z_chunk(ic)

        # ---- attention: j-major scores, 4 key-chunks per 512-wide bank ----
        o_s = act_p.tile([P, CS], bf16, tag="o")
        for h in range(H):
            fo, pb = h // 2, (h % 2) * D
            op = op_ps.tile([P, D + 1], f32, tag="op", name=f"op_{h}")
            for jh in range(2):
                qk = zq_ps.tile([P, 512], f32, tag="zq", name=f"qk_{h}_{jh}")
                for jcl in range(4):
                    jc = jh * 4 + jcl
                    nc.tensor.matmul(
                        qk[:, jcl * P : (jcl + 1) * P],
                        kT_s[pb : pb + D, fo, jc * P : (jc + 1) * P],
                        qT_s[pb : pb + D, fo, :],
                        start=True,
                        stop=True,
                    )
                st = st_p.tile([P, 4, P], f32, tag="st", name=f"st_{h}_{jh}")
                nc.vector.tensor_tensor(
                    st,
                    qk.rearrange("p (a b) -> p a b", a=4),
                    z_s[:, jh * 4 : (jh + 1) * 4, :, h],
                    ALU.add,
                )
                et = et_p.tile([P, 512], bf16, tag="et", name=f"et_{h}_{jh}")
                nc.scalar.activation(et, st, AF.Exp)
                for jcl in range(4):
                    jc = jh * 4 + jcl
                    nc.tensor.matmul(
                        op,
                        et[:, jcl * P : (jcl + 1) * P],
                        v_s[:, jc, h, :],
                        start=(jc == 0),
                        stop=(jc == JC - 1),
                    )
            rec = et_p.tile([P, 1], f32, tag="rec", name=f"rec_{h}")
            nc.vector.reciprocal(rec, op[:, D : D + 1])
            nc.vector.tensor_scalar_mul(o_s[:, h * D : (h + 1) * D], op[:, 0:D], rec)

        # ---- gating + output projection ----
        nc.vector.tensor_mul(g_s, g_s, o_s)
        goT = act_p.tile([P, KC, NI], bf16, tag="goT")
        for gh in range(2):
            tb = big_ps.tile([P, 512], bf16, tag="big", name=f"tb_{gh}")
            for fo in range(gh * 4, gh * 4 + 4):
                nc.tensor.transpose(
                    tb[:, (fo % 4) * P : (fo % 4 + 1) * P],
                    g_s[:, fo * P : (fo + 1) * P],
                    ident,
                )
            nc.vector.tensor_copy(goT[:, gh * 4 : (gh + 1) * 4, :], tb)

        for fh in range(2):
            ps = big_ps.tile([P, 512], f32, tag="big", name=f"op_ps_{fh}")
            for fo in range(KC):
                nc.tensor.matmul(
                    ps,
                    goT[:, fo, :],
                    wo_s[:, fo, fh * 512 : (fh + 1) * 512],
                    start=(fo == 0),
                    stop=(fo == KC - 1),
                )
            out_s = outs_p.tile([P, 512], f32, tag="outs", name=f"out_s{fh}")
            nc.vector.tensor_copy(out_s, ps)
            nc.sync.dma_start(out_d[:, fh * 512 : (fh + 1) * 512], out_s)

    nc.compile()
    return nc


def _chunk128(a):
    # [n*128, m...] -> [128, n, m...] matching rearrange("(co p) m -> p co m")
    n = a.shape[0] // P
    return np.ascontiguousarray(a.reshape(n, P, -1).transpose(1, 0, 2))


def kernel(**inputs):
    global _last_results
    import ml_dtypes
    from concourse.bass_utils import run_bass_kernel_spmd

    bf = ml_dtypes.bfloat16
    s = np.asarray(inputs["s"], dtype=np.float32)[0]
    k_in = np.asarray(inputs["k_in"], dtype=np.float32)[0]
    mask = np.asarray(inputs["mask"], dtype=np.float32)[0]
    bias = np.asarray(inputs["bias"], dtype=np.float32)[0]
    bq = np.asarray(inputs["bq"], dtype=np.float32)
    mult = int(np.asarray(inputs.get("multiplicity", 1)))
    assert mult == 1, f"multiplicity={mult} not supported (B=1)"

    # host-side layout prep (cheap vs device HBM savings)
    sT = _chunk128(s.T.astype(bf))  # [p, co, i_full]
    kinT = _chunk128(k_in.T.astype(bf))  # [p, co, j]
    wT = {
        k: _chunk128(np.asarray(inputs[k], np.float32).T.astype(bf))
        for k in ("Wq", "Wk", "Wv", "Wg", "Wo")
    }
    wz = np.ascontiguousarray(np.asarray(inputs["Wz"], np.float32).astype(bf))
    bq_r = np.ascontiguousarray(bq.reshape(KC, P).T)  # [p, fo] f32
    mask_r = np.ascontiguousarray(mask.reshape(JC, P).T)  # [p, jo] f32
    bias_q = bias.astype(ml_dtypes.float8_e4m3)  # [i_full, j, c]

    nc = _build_program()

    in_maps = []
    for c in range(NCORES):
        # bias^T per core: [c=128, i=128, j=1024]
        biasT = np.ascontiguousarray(
            bias_q[c * NI : (c + 1) * NI].transpose(2, 0, 1)
        )
        in_maps.append(
            {
                "sT": np.ascontiguousarray(sT[:, :, c * NI : (c + 1) * NI]),
                "kinT": kinT,
                "biasT": biasT,
                "wqT": wT["Wq"],
                "wkT": wT["Wk"],
                "wvT": wT["Wv"],
                "wgT": wT["Wg"],
                "woT": wT["Wo"],
                "w_z": wz,
                "b_q": bq_r,
                "mask": mask_r,
            }
        )

    try:
        res = run_bass_kernel_spmd(nc, in_maps, core_ids=list(range(NCORES)))
    except Exception:
        # transient device-unrecoverable errors have been observed on a
        # first attempt; one retry has always succeeded
        import time as _time

        _time.sleep(5.0)
        res = run_bass_kernel_spmd(nc, in_maps, core_ids=list(range(NCORES)))
    _last_results = res
    out = np.concatenate([r["out"] for r in res.results], axis=0)
    return out.reshape(B, I, CS).astype(np.float32)


if __name__ == "__main__":
    rng = np.random.default_rng(0)
    ins = {
        "s": rng.standard_normal((B, I, CS), dtype=np.float32),
        "k_in": rng.standard_normal((B, J, CS), dtype=np.float32),
        "mask": np.ones((B, J), np.float32),
        "bias": rng.standard_normal((B, I, J, CZ), dtype=np.float32),
        "Wq": rng.standard_normal((CS, CS), dtype=np.float32) * 0.02,
        "bq": rng.standard_normal((CS,), dtype=np.float32) * 0.02,
        "Wk": rng.standard_normal((CS, CS), dtype=np.float32) * 0.02,
        "Wv": rng.standard_normal((CS, CS), dtype=np.float32) * 0.02,
        "Wg": rng.standard_normal((CS, CS), dtype=np.float32) * 0.02,
        "Wo": rng.standard_normal((CS, CS), dtype=np.float32) * 0.02,
        "Wz": rng.standard_normal((CZ, H), dtype=np.float32) * 0.02,
        "multiplicity": 1,
    }
    out = kernel(**ins)
    print(out.shape, out.dtype)


# revision 10
# speedup vs baseline: 2.0264x; 1.0706x over previous
# Trainium2 Bass kernel for nn_CrossAttention (B=1, I=J=1024, C_S=1024,
# C_Z=128, H=16, D=64), sharded over the query dim i across 8 NeuronCores.
#
# v4: all large inputs are pre-cast (bias to fp8-e4m3, rest to bf16) AND
# pre-laid-out on the host so every device DMA is a plain contiguous
# [128, N] load (128 big descriptors, full HBM rate, no on-chip transposes):
#   - weights arrive as W^T in [p, co, f] chunk layout; s, k_in transposed
#     likewise; bias arrives as bias^T [c, i, j] per core in e4m3 (the z
#     pair-bias projection tolerates fp8: measured out-relerr 0.006 vs the
#     f64 reference, on top of ~0.005 from the bf16 pipeline).
#   - z[j, i, h] = bias^T Wz via per-(i, jc) stationary matmuls (N=16,
#     fp8 stationary x bf16 moving), interleaved with the projections.
#   - attention is j-major: scores [j, i] accumulate on a DVE-prefilled
#     z slice in PSUM (4 key-chunks per 512-wide bank), one exp per bank,
#     exp output feeds the o-matmul directly as the stationary operand.
#     Softmax denominator comes from an extra masked ones-column on v.
#
# kernel(**inputs) takes FULL inputs, shards on host, runs SPMD on cores 0-7,
# gathers to the full [1, 1024, 1024] output.

import numpy as np

B, I, J, CS, CZ, H, D = 1, 1024, 1024, 1024, 128, 16, 64
NCORES = 8
NI = I // NCORES  # 128 query rows per core
P = 128
KC = CS // P  # 8 contraction chunks
JC = J // P  # 8 key chunks
IC8 = 8  # i rows per bias chunk
NCHUNK = NI // IC8  # 16 bias chunks

_last_results = None


def _build_program():
    from contextlib import ExitStack

    import concourse.mybir as mybir
    import concourse.tile as tile
    from concourse import bacc
    from concourse.masks import make_identity

    f32 = mybir.dt.float32
    bf16 = mybir.dt.bfloat16
    fp8 = mybir.dt.float8e4
    AF = mybir.ActivationFunctionType
    ALU = mybir.AluOpType

    nc = bacc.Bacc("TRN2", target_bir_lowering=False, debug=False)

    # ---- dram io (host-prepared layouts, all partition-major) ----
    sT_d = nc.dram_tensor("sT", [P, KC, NI], bf16, kind="ExternalInput").ap()
    kinT_d = nc.dram_tensor("kinT", [P, KC, J], bf16, kind="ExternalInput").ap()
    biasT_d = nc.dram_tensor("biasT", [P, NI, J], fp8, kind="ExternalInput").ap()
    wqT_d = nc.dram_tensor("wqT", [P, KC, CS], bf16, kind="ExternalInput").ap()
    wkT_d = nc.dram_tensor("wkT", [P, KC, CS], bf16, kind="ExternalInput").ap()
    wvT_d = nc.dram_tensor("wvT", [P, KC, CS], bf16, kind="ExternalInput").ap()
    wgT_d = nc.dram_tensor("wgT", [P, KC, CS], bf16, kind="ExternalInput").ap()
    woT_d = nc.dram_tensor("woT", [P, KC, CS], bf16, kind="ExternalInput").ap()
    wz_d = nc.dram_tensor("w_z", [CZ, H], bf16, kind="ExternalInput").ap()
    bq_d = nc.dram_tensor("b_q", [P, KC], f32, kind="ExternalInput").ap()
    mask_d = nc.dram_tensor("mask", [P, JC], f32, kind="ExternalInput").ap()
    out_d = nc.dram_tensor("out", [NI, CS], f32, kind="ExternalOutput").ap()

    with tile.TileContext(nc) as tc, ExitStack() as ctx:
        pool = lambda name, bufs: ctx.enter_context(tc.tile_pool(name=name, bufs=bufs))
        ppool = lambda name, bufs: ctx.enter_context(
            tc.tile_pool(name=name, bufs=bufs, space="PSUM")
        )

        const = pool("const", 1)
        act_p = pool("act", 1)  # persistent small activations
        big_p = pool("big", 1)  # persistent big tensors (kinT, kT, v, z)
        bstage_p = pool("bstage", 3)  # bias^T chunks
        wstage_p = pool("wstage", 2)  # weight chunks
        et_p = pool("et", 3)
        st_p = pool("st", 2)
        outs_p = pool("outs", 2)

        big_ps = ppool("bigps", 2)  # [128,512] f32: projections / o-proj / go-T
        zq_ps = ppool("zqps", 4)  # [128,512] f32: z accumulation, then qk banks
        op_ps = ppool("ops", 2)  # [128,65] f32: o accumulators

        def copy_on(eng_is_vector, out, in_):
            if eng_is_vector:
                nc.vector.tensor_copy(out, in_)
            else:
                nc.scalar.copy(out, in_)

        # ---- constants / small loads (sync ring) ----
        ident = const.tile([P, P], bf16)
        make_identity(nc, ident)
        wz_s = const.tile([CZ, H], bf16)
        nc.sync.dma_start(wz_s, wz_d)
        bq_s = const.tile([P, KC], f32)
        nc.sync.dma_start(bq_s, bq_d)
        mask_s = const.tile([P, JC], f32)
        nc.sync.dma_start(mask_s, mask_d)
        sT_s = act_p.tile([P, KC, NI], bf16, tag="sT")
        nc.sync.dma_start(sT_s, sT_d)

        def load_w(w_ap, tag):
            w = wstage_p.tile([P, KC, CS], bf16, tag="w", name=tag)
            nc.scalar.dma_start(w, w_ap)
            return w

        # ---- z: bias^T chunks (plain DMA) + per-(i, jc) matmuls ----
        # z_s layout: [j_part, jc, h, i] (bf16) -- i contiguous for the
        # score adds in the attention inner loop
        z_s = big_p.tile([P, JC, H, NI], bf16, tag="z")

        def z_chunk(ic):
            bt = bstage_p.tile([P, IC8, J], fp8, tag="bt", name=f"bt_{ic}")
            nc.sync.dma_start(bt, biasT_d[:, ic * IC8 : (ic + 1) * IC8, :])
            # 4 jc per psum bank: [j=128, (4 jc, 8 i, 16 h)=512]
            for jh in range(2):
                zp = zq_ps.tile([P, 512], f32, tag="zq", name=f"zp_{ic}_{jh}")
                for jcl in range(4):
                    jc = jh * 4 + jcl
                    for il in range(IC8):
                        nc.tensor.matmul(
                            zp[:, jcl * 128 + il * H : jcl * 128 + (il + 1) * H],
                            bt[:, il, jc * P : (jc + 1) * P],
                            wz_s,
                            start=True,
                            stop=True,
                        )
                nc.vector.tensor_copy(
                    z_s[:, jh * 4 : (jh + 1) * 4, :, ic * IC8 : (ic + 1) * IC8],
                    zp.rearrange("p (a b c) -> p a c b", a=4, b=IC8),
                )

        # ---- q projection: qT [f, i] = Wq s^T (+bq, /sqrt(D)) ----
        wq_s = load_w(wqT_d, "wq")  # first on the scalar ring: unlocks q_proj
        kinT_s = big_p.tile([P, KC, J], bf16, tag="kinT")
        nc.scalar.dma_start(kinT_s, kinT_d)
        qT_s = act_p.tile([P, KC, NI], bf16, tag="qT")

        def q_proj():
            for fh in range(2):
                ps = big_ps.tile([P, 512], f32, tag="big", name=f"qp_{fh}")
                for fol in range(4):
                    fo = fh * 4 + fol
                    for co in range(KC):
                        nc.tensor.matmul(
                            ps[:, fol * P : (fol + 1) * P],
                            wq_s[:, co, fo * P : (fo + 1) * P],
                            sT_s[:, co, :],
                            start=(co == 0),
                            stop=(co == KC - 1),
                        )
                for fol in range(4):
                    fo = fh * 4 + fol
                    nc.vector.tensor_scalar(
                        qT_s[:, fo, :],
                        ps[:, fol * P : (fol + 1) * P],
                        bq_s[:, fo : fo + 1],
                        1.0 / np.sqrt(D),
                        ALU.add,
                        ALU.mult,
                    )

        z_chunk(0)
        z_chunk(1)
        q_proj()
        z_chunk(2)
        z_chunk(3)

        # ---- k projection: kT [f, j] = Wk k_in^T ----
        wk_s = load_w(wkT_d, "wk")
        kT_s = big_p.tile([P, KC, J], bf16, tag="kT")
        for fo in range(KC):
            for jh in range(2):
                ps = big_ps.tile([P, 512], f32, tag="big", name=f"kp_{fo}_{jh}")
                for co in range(KC):
                    nc.tensor.matmul(
                        ps,
                        wk_s[:, co, fo * P : (fo + 1) * P],
                        kinT_s[:, co, jh * 512 : (jh + 1) * 512],
                        start=(co == 0),
                        stop=(co == KC - 1),
                    )
                copy_on(jh == 0, kT_s[:, fo, jh * 512 : (jh + 1) * 512], ps)
            if fo % 2 == 1:
                z_chunk(4 + fo // 2)

        # ---- v projection: v [j, h, d|mask] = k_in Wv^T, masked ----
        wv_s = load_w(wvT_d, "wv")
        v_s = big_p.tile([P, JC, H, D + 1], bf16, tag="v")
        for jo in range(JC):
            for fh in range(2):
                ps = big_ps.tile([P, 512], f32, tag="big", name=f"vp_{jo}_{fh}")
                for co in range(KC):
                    nc.tensor.matmul(
                        ps,
                        kinT_s[:, co, jo * P : (jo + 1) * P],
                        wv_s[:, co, fh * 512 : (fh + 1) * 512],
                        start=(co == 0),
                        stop=(co == KC - 1),
                    )
                nc.vector.tensor_scalar_mul(
                    v_s[:, jo, fh * 8 : (fh + 1) * 8, 0:D],
                    ps,
                    mask_s[:, jo : jo + 1],
                )
            nc.vector.tensor_copy(
                v_s[:, jo, :, D : D + 1],
                mask_s[:, jo : jo + 1, None].to_broadcast((P, H, 1)),
            )
            if jo % 2 == 1:
                z_chunk(8 + jo // 2)

        # ---- g projection: g [i, f] = sigmoid(s Wg^T) ----
        wg_s = load_w(wgT_d, "wg")
        g_s = act_p.tile([P, CS], bf16, tag="g")
        for fh in range(2):
            ps = big_ps.tile([P, 512], f32, tag="big", name=f"gp_{fh}")
            for co in range(KC):
                nc.tensor.matmul(
                    ps,
                    sT_s[:, co, :],
                    wg_s[:, co, fh * 512 : (fh + 1) * 512],
                    start=(co == 0),
                    stop=(co == KC - 1),
                )
            nc.scalar.activation(g_s[:, fh * 512 : (fh + 1) * 512], ps, AF.Sigmoid)
            z_chunk(12 + fh)

        wo_s = load_w(woT_d, "wo")
        for ic in range(14, NCHUNK):
            z_chunk(ic)

        # ---- attention: j-major scores, 4 key-chunks per 512-wide bank ----
        o_s = act_p.tile([P, CS], bf16, tag="o")
        goT = act_p.tile([P, KC, NI], bf16, tag="goT")
        for h in range(H):
            fo, pb = h // 2, (h % 2) * D
            op = op_ps.tile([P, D + 1], f32, tag="op", name=f"op_{h}")
            for jh in range(2):
                qk = zq_ps.tile([P, 512], f32, tag="zq", name=f"qk_{h}_{jh}")
                for jcl in range(4):
                    jc = jh * 4 + jcl
                    nc.tensor.matmul(
                        qk[:, jcl * P : (jcl + 1) * P],
                        kT_s[pb : pb + D, fo, jc * P : (jc + 1) * P],
                        qT_s[pb : pb + D, fo, :],
                        start=True,
                        stop=True,
                    )
                st = st_p.tile([P, 4, P], f32, tag="st", name=f"st_{h}_{jh}")
                nc.vector.tensor_tensor(
                    st,
                    qk.rearrange("p (a b) -> p a b", a=4),
                    z_s[:, jh * 4 : (jh + 1) * 4, h, :],
                    ALU.add,
                )
                et = et_p.tile([P, 512], bf16, tag="et", name=f"et_{h}_{jh}")
                nc.scalar.activation(et, st, AF.Exp)
                for jcl in range(4):
                    jc = jh * 4 + jcl
                    nc.tensor.matmul(
                        op,
                        et[:, jcl * P : (jcl + 1) * P],
                        v_s[:, jc, h, :],
                        start=(jc == 0),
                        stop=(jc == JC - 1),
                    )
            rec = et_p.tile([P, 1], f32, tag="rec", name=f"rec_{h}")
            nc.vector.reciprocal(rec, op[:, D : D + 1])
            nc.vector.tensor_scalar_mul(o_s[:, h * D : (h + 1) * D], op[:, 0:D], rec)
            if h % 8 == 7:
                # gate + transpose this half while later heads proceed
                gh = h // 8
                nc.vector.tensor_mul(
                    g_s[:, gh * 512 : (gh + 1) * 512],
                    g_s[:, gh * 512 : (gh + 1) * 512],
                    o_s[:, gh * 512 : (gh + 1) * 512],
                )
                tb = big_ps.tile([P, 512], bf16, tag="big", name=f"tb_{gh}")
                for fo in range(gh * 4, gh * 4 + 4):
                    nc.tensor.transpose(
                        tb[:, (fo % 4) * P : (fo % 4 + 1) * P],
                        g_s[:, fo * P : (fo + 1) * P],
                        ident,
                    )
                nc.vector.tensor_copy(goT[:, gh * 4 : (gh + 1) * 4, :], tb)

        for fh in range(2):
            ps = big_ps.tile([P, 512], f32, tag="big", name=f"op_ps_{fh}")
            for fo in range(KC):
                nc.tensor.matmul(
                    ps,
                    goT[:, fo, :],
                    wo_s[:, fo, fh * 512 : (fh + 1) * 512],
                    start=(fo == 0),
                    stop=(fo == KC - 1),
                )
            out_s = outs_p.tile([P, 512], f32, tag="outs", name=f"out_s{fh}")
            nc.scalar.copy(out_s, ps)
            nc.sync.dma_start(out_d[:, fh * 512 : (fh + 1) * 512], out_s)

    nc.compile()
    return nc


def _chunk128(a):
    # [n*128, m...] -> [128, n, m...] matching rearrange("(co p) m -> p co m")
    n = a.shape[0] // P
    return np.ascontiguousarray(a.reshape(n, P, -1).transpose(1, 0, 2))


def kernel(**inputs):
    global _last_results
    import ml_dtypes
    from concourse.bass_utils import run_bass_kernel_spmd

    bf = ml_dtypes.bfloat16
    s = np.asarray(inputs["s"], dtype=np.float32)[0]
    k_in = np.asarray(inputs["k_in"], dtype=np.float32)[0]
    mask = np.asarray(inputs["mask"], dtype=np.float32)[0]
    bias = np.asarray(inputs["bias"], dtype=np.float32)[0]
    bq = np.asarray(inputs["bq"], dtype=np.float32)
    mult = int(np.asarray(inputs.get("multiplicity", 1)))
    assert mult == 1, f"multiplicity={mult} not supported (B=1)"

    # host-side layout prep (cheap vs device HBM savings)
    sT = _chunk128(s.T.astype(bf))  # [p, co, i_full]
    kinT = _chunk128(k_in.T.astype(bf))  # [p, co, j]
    wT = {
        k: _chunk128(np.asarray(inputs[k], np.float32).T.astype(bf))
        for k in ("Wq", "Wk", "Wv", "Wg", "Wo")
    }
    wz = np.ascontiguousarray(np.asarray(inputs["Wz"], np.float32).astype(bf))
    bq_r = np.ascontiguousarray(bq.reshape(KC, P).T)  # [p, fo] f32
    mask_r = np.ascontiguousarray(mask.reshape(JC, P).T)  # [p, jo] f32
    bias_q = bias.astype(ml_dtypes.float8_e4m3)  # [i_full, j, c]

    nc = _build_program()

    in_maps = []
    for c in range(NCORES):
        # bias^T per core: [c=128, i=128, j=1024]
        biasT = np.ascontiguousarray(
            bias_q[c * NI : (c + 1) * NI].transpose(2, 0, 1)
        )
        in_maps.append(
            {
                "sT": np.ascontiguousarray(sT[:, :, c * NI : (c + 1) * NI]),
                "kinT": kinT,
                "biasT": biasT,
                "wqT": wT["Wq"],
                "wkT": wT["Wk"],
                "wvT": wT["Wv"],
                "wgT": wT["Wg"],
                "woT": wT["Wo"],
                "w_z": wz,
                "b_q": bq_r,
                "mask": mask_r,
            }
        )

    try:
        res = run_bass_kernel_spmd(nc, in_maps, core_ids=list(range(NCORES)))
    except Exception:
        # transient device-unrecoverable errors have been observed on a
        # first attempt; one retry has always succeeded
        import time as _time

        _time.sleep(5.0)
        res = run_bass_kernel_spmd(nc, in_maps, core_ids=list(range(NCORES)))
    _last_results = res
    out = np.concatenate([r["out"] for r in res.results], axis=0)
    return out.reshape(B, I, CS).astype(np.float32)


if __name__ == "__main__":
    rng = np.random.default_rng(0)
    ins = {
        "s": rng.standard_normal((B, I, CS), dtype=np.float32),
        "k_in": rng.standard_normal((B, J, CS), dtype=np.float32),
        "mask": np.ones((B, J), np.float32),
        "bias": rng.standard_normal((B, I, J, CZ), dtype=np.float32),
        "Wq": rng.standard_normal((CS, CS), dtype=np.float32) * 0.02,
        "bq": rng.standard_normal((CS,), dtype=np.float32) * 0.02,
        "Wk": rng.standard_normal((CS, CS), dtype=np.float32) * 0.02,
        "Wv": rng.standard_normal((CS, CS), dtype=np.float32) * 0.02,
        "Wg": rng.standard_normal((CS, CS), dtype=np.float32) * 0.02,
        "Wo": rng.standard_normal((CS, CS), dtype=np.float32) * 0.02,
        "Wz": rng.standard_normal((CZ, H), dtype=np.float32) * 0.02,
        "multiplicity": 1,
    }
    out = kernel(**ins)
    print(out.shape, out.dtype)


# revision 11
# speedup vs baseline: 2.0601x; 1.0166x over previous
# Trainium2 Bass kernel for nn_CrossAttention (B=1, I=J=1024, C_S=1024,
# C_Z=128, H=16, D=64), sharded over the query dim i across 8 NeuronCores.
#
# v4: all large inputs are pre-cast (bias to fp8-e4m3, rest to bf16) AND
# pre-laid-out on the host so every device DMA is a plain contiguous
# [128, N] load (128 big descriptors, full HBM rate, no on-chip transposes):
#   - weights arrive as W^T in [p, co, f] chunk layout; s, k_in transposed
#     likewise; bias arrives as bias^T [c, i, j] per core in e4m3 (the z
#     pair-bias projection tolerates fp8: measured out-relerr 0.006 vs the
#     f64 reference, on top of ~0.005 from the bf16 pipeline).
#   - z[j, i, h] = bias^T Wz via per-(i, jc) stationary matmuls (N=16,
#     fp8 stationary x bf16 moving), interleaved with the projections.
#   - attention is j-major: scores [j, i] accumulate on a DVE-prefilled
#     z slice in PSUM (4 key-chunks per 512-wide bank), one exp per bank,
#     exp output feeds the o-matmul directly as the stationary operand.
#     Softmax denominator comes from an extra masked ones-column on v.
#
# kernel(**inputs) takes FULL inputs, shards on host, runs SPMD on cores 0-7,
# gathers to the full [1, 1024, 1024] output.

import numpy as np

B, I, J, CS, CZ, H, D = 1, 1024, 1024, 1024, 128, 16, 64
NCORES = 8
NI = I // NCORES  # 128 query rows per core
P = 128
KC = CS // P  # 8 contraction chunks
JC = J // P  # 8 key chunks
IC8 = 4  # i rows per bias chunk
NCHUNK = NI // IC8  # 32 bias chunks

_last_results = None


def _build_program():
    from contextlib import ExitStack

    import concourse.mybir as mybir
    import concourse.tile as tile
    from concourse import bacc
    from concourse.masks import make_identity

    f32 = mybir.dt.float32
    bf16 = mybir.dt.bfloat16
    fp8 = mybir.dt.float8e4
    AF = mybir.ActivationFunctionType
    ALU = mybir.AluOpType

    nc = bacc.Bacc("TRN2", target_bir_lowering=False, debug=False)

    # ---- dram io (host-prepared layouts, all partition-major) ----
    sT_d = nc.dram_tensor("sT", [P, KC, NI], bf16, kind="ExternalInput").ap()
    kinT_d = nc.dram_tensor("kinT", [P, KC, J], bf16, kind="ExternalInput").ap()
    biasT_d = nc.dram_tensor("biasT", [P, NI, J], fp8, kind="ExternalInput").ap()
    wqT_d = nc.dram_tensor("wqT", [P, KC, CS], bf16, kind="ExternalInput").ap()
    wkT_d = nc.dram_tensor("wkT", [P, KC, CS], bf16, kind="ExternalInput").ap()
    wvT_d = nc.dram_tensor("wvT", [P, KC, CS], bf16, kind="ExternalInput").ap()
    wgT_d = nc.dram_tensor("wgT", [P, KC, CS], bf16, kind="ExternalInput").ap()
    woT_d = nc.dram_tensor("woT", [P, KC, CS], bf16, kind="ExternalInput").ap()
    wz_d = nc.dram_tensor("w_z", [CZ, H], bf16, kind="ExternalInput").ap()
    bq_d = nc.dram_tensor("b_q", [P, KC], f32, kind="ExternalInput").ap()
    mask_d = nc.dram_tensor("mask", [P, JC], f32, kind="ExternalInput").ap()
    out_d = nc.dram_tensor("out", [NI, CS], f32, kind="ExternalOutput").ap()

    with tile.TileContext(nc) as tc, ExitStack() as ctx:
        pool = lambda name, bufs: ctx.enter_context(tc.tile_pool(name=name, bufs=bufs))
        ppool = lambda name, bufs: ctx.enter_context(
            tc.tile_pool(name=name, bufs=bufs, space="PSUM")
        )

        const = pool("const", 1)
        act_p = pool("act", 1)  # persistent small activations
        big_p = pool("big", 1)  # persistent big tensors (kinT, kT, v, z)
        bstage_p = pool("bstage", 4)  # bias^T chunks
        wstage_p = pool("wstage", 2)  # weight chunks
        et_p = pool("et", 3)
        st_p = pool("st", 2)
        outs_p = pool("outs", 2)

        big_ps = ppool("bigps", 2)  # [128,512] f32: projections / o-proj / go-T
        zq_ps = ppool("zqps", 4)  # [128,512] f32: z accumulation, then qk banks
        op_ps = ppool("ops", 2)  # [128,65] f32: o accumulators

        def copy_on(eng_is_vector, out, in_):
            if eng_is_vector:
                nc.vector.tensor_copy(out, in_)
            else:
                nc.scalar.copy(out, in_)

        # ---- constants / small loads (sync ring) ----
        ident = const.tile([P, P], bf16)
        make_identity(nc, ident)
        wz_s = const.tile([CZ, H], bf16)
        nc.sync.dma_start(wz_s, wz_d)

        def load_w(w_ap, tag):
            w = wstage_p.tile([P, KC, CS], bf16, tag="w", name=tag)
            nc.scalar.dma_start(w, w_ap)
            return w

        # ---- z: bias^T chunks (plain DMA) + per-(i, jc) matmuls ----
        # z_s layout: [j_part, jc, h, i] (bf16) -- i contiguous for the
        # score adds in the attention inner loop
        z_s = big_p.tile([P, JC, H, NI], bf16, tag="z")

        def z_chunk(ic):
            bt = bstage_p.tile([P, IC8, J], fp8, tag="bt", name=f"bt_{ic}")
            nc.sync.dma_start(bt, biasT_d[:, ic * IC8 : (ic + 1) * IC8, :])
            # all 8 jc in one psum bank: [j=128, (8 jc, 4 i, 16 h)=512]
            zp = zq_ps.tile([P, 512], f32, tag="zq", name=f"zp_{ic}")
            for jc in range(JC):
                for il in range(IC8):
                    nc.tensor.matmul(
                        zp[:, jc * 64 + il * H : jc * 64 + (il + 1) * H],
                        bt[:, il, jc * P : (jc + 1) * P],
                        wz_s,
                        start=True,
                        stop=True,
                    )
            nc.vector.tensor_copy(
                z_s[:, :, :, ic * IC8 : (ic + 1) * IC8],
                zp.rearrange("p (a b c) -> p a c b", a=JC, b=IC8),
            )

        # ---- q projection: qT [f, i] = Wq s^T (+bq, /sqrt(D)) ----
        wq_s = load_w(wqT_d, "wq")  # first on the scalar ring: unlocks q_proj
        qT_s = act_p.tile([P, KC, NI], bf16, tag="qT")

        def q_proj():
            for fh in range(2):
                ps = big_ps.tile([P, 512], f32, tag="big", name=f"qp_{fh}")
                for fol in range(4):
                    fo = fh * 4 + fol
                    for co in range(KC):
                        nc.tensor.matmul(
                            ps[:, fol * P : (fol + 1) * P],
                            wq_s[:, co, fo * P : (fo + 1) * P],
                            sT_s[:, co, :],
                            start=(co == 0),
                            stop=(co == KC - 1),
                        )
                for fol in range(4):
                    fo = fh * 4 + fol
                    nc.vector.tensor_scalar(
                        qT_s[:, fo, :],
                        ps[:, fol * P : (fol + 1) * P],
                        bq_s[:, fo : fo + 1],
                        1.0 / np.sqrt(D),
                        ALU.add,
                        ALU.mult,
                    )

        z_chunk(0)
        z_chunk(1)
        z_chunk(2)
        sT_s = act_p.tile([P, KC, NI], bf16, tag="sT")
        nc.sync.dma_start(sT_s, sT_d)
        kinT_s = big_p.tile([P, KC, J], bf16, tag="kinT")
        nc.sync.dma_start(kinT_s, kinT_d)
        bq_s = const.tile([P, KC], f32)
        nc.sync.dma_start(bq_s, bq_d)
        mask_s = const.tile([P, JC], f32)
        nc.sync.dma_start(mask_s, mask_d)
        z_chunk(3)
        q_proj()
        z_chunk(4)
        z_chunk(5)

        # ---- k projection: kT [f, j] = Wk k_in^T ----
        wk_s = load_w(wkT_d, "wk")
        kT_s = big_p.tile([P, KC, J], bf16, tag="kT")
        for fo in range(KC):
            for jh in range(2):
                ps = big_ps.tile([P, 512], f32, tag="big", name=f"kp_{fo}_{jh}")
                for co in range(KC):
                    nc.tensor.matmul(
                        ps,
                        wk_s[:, co, fo * P : (fo + 1) * P],
                        kinT_s[:, co, jh * 512 : (jh + 1) * 512],
                        start=(co == 0),
                        stop=(co == KC - 1),
                    )
                copy_on(jh == 0, kT_s[:, fo, jh * 512 : (jh + 1) * 512], ps)
            z_chunk(6 + fo)

        # ---- v projection: v [j, h, d|mask] = k_in Wv^T, masked ----
        wv_s = load_w(wvT_d, "wv")
        v_s = big_p.tile([P, JC, H, D + 1], bf16, tag="v")
        for jo in range(JC):
            for fh in range(2):
                ps = big_ps.tile([P, 512], f32, tag="big", name=f"vp_{jo}_{fh}")
                for co in range(KC):
                    nc.tensor.matmul(
                        ps,
                        kinT_s[:, co, jo * P : (jo + 1) * P],
                        wv_s[:, co, fh * 512 : (fh + 1) * 512],
                        start=(co == 0),
                        stop=(co == KC - 1),
                    )
                nc.vector.tensor_scalar_mul(
                    v_s[:, jo, fh * 8 : (fh + 1) * 8, 0:D],
                    ps,
                    mask_s[:, jo : jo + 1],
                )
            nc.vector.tensor_copy(
                v_s[:, jo, :, D : D + 1],
                mask_s[:, jo : jo + 1, None].to_broadcast((P, H, 1)),
            )
            z_chunk(14 + jo)

        # ---- g projection: g [i, f] = sigmoid(s Wg^T) ----
        wg_s = load_w(wgT_d, "wg")
        g_s = act_p.tile([P, CS], bf16, tag="g")
        for fh in range(2):
            ps = big_ps.tile([P, 512], f32, tag="big", name=f"gp_{fh}")
            for co in range(KC):
                nc.tensor.matmul(
                    ps,
                    sT_s[:, co, :],
                    wg_s[:, co, fh * 512 : (fh + 1) * 512],
                    start=(co == 0),
                    stop=(co == KC - 1),
                )
            nc.scalar.activation(g_s[:, fh * 512 : (fh + 1) * 512], ps, AF.Sigmoid)
            z_chunk(22 + 2 * fh)
            z_chunk(23 + 2 * fh)

        wo_s = load_w(woT_d, "wo")
        for ic in range(26, NCHUNK):
            z_chunk(ic)

        # ---- attention: j-major scores, 4 key-chunks per 512-wide bank ----
        o_s = act_p.tile([P, CS], bf16, tag="o")
        goT = act_p.tile([P, KC, NI], bf16, tag="goT")
        for h in range(H):
            fo, pb = h // 2, (h % 2) * D
            op = op_ps.tile([P, D + 1], f32, tag="op", name=f"op_{h}")
            for jh in range(2):
                qk = zq_ps.tile([P, 512], f32, tag="zq", name=f"qk_{h}_{jh}")
                for jcl in range(4):
                    jc = jh * 4 + jcl
                    nc.tensor.matmul(
                        qk[:, jcl * P : (jcl + 1) * P],
                        kT_s[pb : pb + D, fo, jc * P : (jc + 1) * P],
                        qT_s[pb : pb + D, fo, :],
                        start=True,
                        stop=True,
                    )
                st = st_p.tile([P, 4, P], f32, tag="st", name=f"st_{h}_{jh}")
                nc.vector.tensor_tensor(
                    st,
                    qk.rearrange("p (a b) -> p a b", a=4),
                    z_s[:, jh * 4 : (jh + 1) * 4, h, :],
                    ALU.add,
                )
                et = et_p.tile([P, 512], bf16, tag="et", name=f"et_{h}_{jh}")
                nc.scalar.activation(et, st, AF.Exp)
                for jcl in range(4):
                    jc = jh * 4 + jcl
                    nc.tensor.matmul(
                        op,
                        et[:, jcl * P : (jcl + 1) * P],
                        v_s[:, jc, h, :],
                        start=(jc == 0),
                        stop=(jc == JC - 1),
                    )
            rec = et_p.tile([P, 1], f32, tag="rec", name=f"rec_{h}")
            nc.vector.reciprocal(rec, op[:, D : D + 1])
            nc.vector.tensor_scalar_mul(o_s[:, h * D : (h + 1) * D], op[:, 0:D], rec)
            if h % 8 == 7:
                # gate + transpose this half while later heads proceed
                gh = h // 8
                nc.vector.tensor_mul(
                    g_s[:, gh * 512 : (gh + 1) * 512],
                    g_s[:, gh * 512 : (gh + 1) * 512],
                    o_s[:, gh * 512 : (gh + 1) * 512],
                )
                tb = big_ps.tile([P, 512], bf16, tag="big", name=f"tb_{gh}")
                for fo in range(gh * 4, gh * 4 + 4):
                    nc.tensor.transpose(
                        tb[:, (fo % 4) * P : (fo % 4 + 1) * P],
                        g_s[:, fo * P : (fo + 1) * P],
                        ident,
                    )
                nc.vector.tensor_copy(goT[:, gh * 4 : (gh + 1) * 4, :], tb)

        for fh in range(2):
            ps = big_ps.tile([P, 512], f32, tag="big", name=f"op_ps_{fh}")
            for fo in range(KC):
                nc.tensor.matmul(
                    ps,
                    goT[:, fo, :],
                    wo_s[:, fo, fh * 512 : (fh + 1) * 512],
                    start=(fo == 0),
                    stop=(fo == KC - 1),
                )
            out_s = outs_p.tile([P, 512], f32, tag="outs", name=f"out_s{fh}")
            nc.scalar.copy(out_s, ps)
            nc.sync.dma_start(out_d[:, fh * 512 : (fh + 1) * 512], out_s)

    nc.compile()
    return nc


def _chunk128(a):
    # [n*128, m...] -> [128, n, m...] matching rearrange("(co p) m -> p co m")
    n = a.shape[0] // P
    return np.ascontiguousarray(a.reshape(n, P, -1).transpose(1, 0, 2))


def kernel(**inputs):
    global _last_results
    import ml_dtypes
    from concourse.bass_utils import run_bass_kernel_spmd

    bf = ml_dtypes.bfloat16
    s = np.asarray(inputs["s"], dtype=np.float32)[0]
    k_in = np.asarray(inputs["k_in"], dtype=np.float32)[0]
    mask = np.asarray(inputs["mask"], dtype=np.float32)[0]
    bias = np.asarray(inputs["bias"], dtype=np.float32)[0]
    bq = np.asarray(inputs["bq"], dtype=np.float32)
    mult = int(np.asarray(inputs.get("multiplicity", 1)))
    assert mult == 1, f"multiplicity={mult} not supported (B=1)"

    # host-side layout prep (cheap vs device HBM savings)
    sT = _chunk128(s.T.astype(bf))  # [p, co, i_full]
    kinT = _chunk128(k_in.T.astype(bf))  # [p, co, j]
    wT = {
        k: _chunk128(np.asarray(inputs[k], np.float32).T.astype(bf))
        for k in ("Wq", "Wk", "Wv", "Wg", "Wo")
    }
    wz = np.ascontiguousarray(np.asarray(inputs["Wz"], np.float32).astype(bf))
    bq_r = np.ascontiguousarray(bq.reshape(KC, P).T)  # [p, fo] f32
    mask_r = np.ascontiguousarray(mask.reshape(JC, P).T)  # [p, jo] f32
    bias_q = bias.astype(ml_dtypes.float8_e4m3)  # [i_full, j, c]

    nc = _build_program()

    in_maps = []
    for c in range(NCORES):
        # bias^T per core: [c=128, i=128, j=1024]
        biasT = np.ascontiguousarray(
            bias_q[c * NI : (c + 1) * NI].transpose(2, 0, 1)
        )
        in_maps.append(
            {
                "sT": np.ascontiguousarray(sT[:, :, c * NI : (c + 1) * NI]),
                "kinT": kinT,
                "biasT": biasT,
                "wqT": wT["Wq"],
                "wkT": wT["Wk"],
                "wvT": wT["Wv"],
                "wgT": wT["Wg"],
                "woT": wT["Wo"],
                "w_z": wz,
                "b_q": bq_r,
                "mask": mask_r,
            }
        )

    try:
        res = run_bass_kernel_spmd(nc, in_maps, core_ids=list(range(NCORES)))
    except Exception:
        # transient device-unrecoverable errors have been observed on a
        # first attempt; one retry has always succeeded
        import time as _time

        _time.sleep(5.0)
        res = run_bass_kernel_spmd(nc, in_maps, core_ids=list(range(NCORES)))
    _last_results = res
    out = np.concatenate([r["out"] for r in res.results], axis=0)
    return out.reshape(B, I, CS).astype(np.float32)


if __name__ == "__main__":
    rng = np.random.default_rng(0)
    ins = {
        "s": rng.standard_normal((B, I, CS), dtype=np.float32),
        "k_in": rng.standard_normal((B, J, CS), dtype=np.float32),
        "mask": np.ones((B, J), np.float32),
        "bias": rng.standard_normal((B, I, J, CZ), dtype=np.float32),
        "Wq": rng.standard_normal((CS, CS), dtype=np.float32) * 0.02,
        "bq": rng.standard_normal((CS,), dtype=np.float32) * 0.02,
        "Wk": rng.standard_normal((CS, CS), dtype=np.float32) * 0.02,
        "Wv": rng.standard_normal((CS, CS), dtype=np.float32) * 0.02,
        "Wg": rng.standard_normal((CS, CS), dtype=np.float32) * 0.02,
        "Wo": rng.standard_normal((CS, CS), dtype=np.float32) * 0.02,
        "Wz": rng.standard_normal((CZ, H), dtype=np.float32) * 0.02,
        "multiplicity": 1,
    }
    out = kernel(**ins)
    print(out.shape, out.dtype)
